# revision 32
# baseline (speedup 1.0000x reference)
"""Trainium2 Bass kernel for DiceLoss (hard-argmax dice, ignore background, mean).

Problem (hardcoded shapes):
  y_true: [16, 512, 512] int32 in [0, 8)
  y_pred: [16, 8, 512, 512] float32
  out   : scalar float32 = mean over classes 1..7 of
          (2*tp + eps) / (2*tp + fp + fn + eps)
  with pred_cls = argmax_c y_pred, one-hot tp/fp/fn sums over all pixels.
  Note 2*tp + fp + fn == pred_cnt + gt_cnt, so per class we only need
  tp, pred_cnt (both from the device) and gt_cnt (host bincount of y_true).

Strategy (8 NeuronCores, data-parallel over batch; measured-on-HW numbers in
brackets):
  - Each core processes 2 of the 16 batch images (SPMD, same NEFF), streamed
    in [128, 1024] chunks; the final chunk is split in two [128, 512] halves
    to shorten the post-DMA tail.
  - ScalarE converts the 8 channel planes fp32->fp16 and the label plane
    int32->fp16 [1.15us per [128,1024] op, no 16-bit speedup]. GpSimd is NOT
    used: it shares the DVE datapath and stalls DVE 2x/4x ops (measured).
  - VectorE (DVE): everything fp16 to hit the hardware perf modes
    [tensor_tensor 16-bit in+out runs 2x (~600ns); tensor_scalar 16-bit
    without accum_out runs 4x (~335ns); accum_out forces 1x so is avoided;
    scalar_tensor_tensor is always 1x so is avoided]:
      * 7-op pairwise-max tree over the fp16 channels -> m
      * pred_c = tensor_tensor is_equal(ch_fp16[c], m)   (2x)
      * gt_c   = tensor_scalar is_equal(lab_fp16, c)     (4x)
    fp16 keeps argmax-tie inflation ~0.05% of pixels (rel err ~3e-4,
    tolerance is 2e-2).
  - TensorE (PE): per class, tp via diag(pred_c^T @ gt_c) accumulated in a
    [128,128] PSUM bank over subtiles+chunks, plus pred_cnt via an extra
    rhs=ones[128,1] matmul on the already-loaded pred_c weights [LDWEIGHTS
    and MATMUL pipeline on separate units]. The 7 pred-count accumulators
    share the 8th PSUM bank; start=True resets the WHOLE bank (measured),
    so only the very first count matmul sets it.
  - Host: gt counts from np.bincount(y_true) (exact, input-only), then the
    dice mean in float32 mirroring the reference arithmetic.
"""

import numpy as np

EPS = 1e-05

# Problem geometry (hardcoded per the harness contract).
N_CORES = 8
NB = 2          # batch images per core
C = 8           # classes
P = 128         # SBUF partitions
F = 1024        # free-dim elements per chunk
NCHUNK = 2      # chunks per image plane (512*512 = 2*128*1024)

_CACHED_NC = None


def build_bass():
    """Build the Bass kernel (same NEFF for all 8 cores)."""
    from contextlib import ExitStack

    import concourse.bacc as bacc
    import concourse.tile as tile
    from concourse import mybir

    nc = bacc.Bacc(None, target_bir_lowering=False)
    f32 = mybir.dt.float32
    fp16 = mybir.dt.float16
    i32 = mybir.dt.int32
    A = mybir.AluOpType

    yp = nc.dram_tensor(
        "yp", [NB, C, NCHUNK, P, F], f32, kind="ExternalInput"
    )
    yt = nc.dram_tensor("yt", [NB, NCHUNK, P, F], i32, kind="ExternalInput")
    # tp partials: per class a [128, 128] PSUM accumulator; host takes trace().
    tp_out = nc.dram_tensor("tp_out", [7, P, 128], f32, kind="ExternalOutput")
    # pred counts: per class a [128, 1] PSUM accumulator; host sums partitions.
    pc_out = nc.dram_tensor("pc_out", [P, 7], f32, kind="ExternalOutput")

    # pieces: (n, j, lo, hi). The first chunk ramps up in small pieces (the
    # DMA round-robins a piece's planes, so the first convert can only start
    # once ~a whole piece has landed); the final chunk is split for a shorter
    # post-DMA tail.
    pieces = []
    for n in range(NB):
        for j in range(NCHUNK):
            if n == 0 and j == 0:
                pieces.append((n, j, 0, 256))
                pieces.append((n, j, 256, 512))
                pieces.append((n, j, 512, F))
            elif n == NB - 1 and j == NCHUNK - 1:
                pieces.append((n, j, 0, F // 2))
                pieces.append((n, j, F // 2, F))
            else:
                pieces.append((n, j, 0, F))

    with tile.TileContext(nc) as tc, ExitStack() as ctx:
        chpool = ctx.enter_context(tc.tile_pool(name="ch", bufs=2))
        hpool = ctx.enter_context(tc.tile_pool(name="h", bufs=2))
        tpool = ctx.enter_context(tc.tile_pool(name="tt", bufs=2))
        mpool = ctx.enter_context(tc.tile_pool(name="mx", bufs=2))
        mtmp = ctx.enter_context(tc.tile_pool(name="mtmp", bufs=6))
        maskp = ctx.enter_context(tc.tile_pool(name="mask", bufs=3))
        gtpool = ctx.enter_context(tc.tile_pool(name="gt", bufs=9))
        constp = ctx.enter_context(tc.tile_pool(name="const", bufs=1))
        accp = ctx.enter_context(tc.tile_pool(name="acc", bufs=1))
        psump = ctx.enter_context(tc.tile_pool(name="psum", bufs=1, space="PSUM"))

        ones = constp.tile([P, 1], fp16, name="ones")
        nc.vector.memset(ones, 1.0)

        psums = [
            psump.tile([P, 128], f32, name=f"ps{c}", tag=f"ps{c}")
            for c in range(1, C)
        ]
        # all 7 pred-count accumulators share one PSUM bank (disjoint columns)
        cntbank = psump.tile([P, 8], f32, name="cntbank", tag="cntbank")
        cnts = [cntbank[:, c - 1 : c] for c in range(1, C)]

        npieces = len(pieces)
        for pi, (n, j, lo, hi) in enumerate(pieces):
            W = hi - lo
            nsub = W // 128
            # label DMA first so its convert (also first on ScalarE) never
            # stalls; the gt masks then run on DVE while channels stream
            tt = tpool.tile([P, W], i32, name="t", tag="t")
            nc.sync.dma_start(out=tt, in_=yt[n, j][:, lo:hi])
            ch = []
            for c in range(C):
                tl = chpool.tile([P, W], f32, name=f"ch{c}", tag=f"ch{c}")
                nc.sync.dma_start(out=tl, in_=yp[n, c, j][:, lo:hi])
                ch.append(tl)

            tf = tpool.tile([P, W], fp16, name="tf", tag="tf")
            nc.scalar.copy(out=tf, in_=tt)
            chf = []
            for c in range(C):
                tl = hpool.tile([P, W], fp16, name=f"hf{c}", tag=f"hf{c}")
                nc.scalar.copy(out=tl, in_=ch[c])
                chf.append(tl)

            # gt masks early: only need the labels (tensor_scalar 4x mode)
            gts = {}
            for c in range(1, C):
                gt = gtpool.tile([P, W], fp16, name=f"gt{c}", tag="gt")
                nc.vector.tensor_scalar(gt, tf, float(c), None, A.is_equal)
                gts[c] = gt

            # ---- max tree (DVE tensor_tensor fp16: 2x perf mode); ordered
            # so only m67 -> m4567 -> m depend on the last channel ----
            m01 = mtmp.tile([P, W], fp16, name="m01", tag="mt")
            nc.vector.tensor_max(m01, chf[0], chf[1])
            m23 = mtmp.tile([P, W], fp16, name="m23", tag="mt")
            nc.vector.tensor_max(m23, chf[2], chf[3])
            m0123 = mtmp.tile([P, W], fp16, name="m0123", tag="mt")
            nc.vector.tensor_max(m0123, m01, m23)
            m45 = mtmp.tile([P, W], fp16, name="m45", tag="mt")
            nc.vector.tensor_max(m45, chf[4], chf[5])
            m67 = mtmp.tile([P, W], fp16, name="m67", tag="mt")
            nc.vector.tensor_max(m67, chf[6], chf[7])
            m4567 = mtmp.tile([P, W], fp16, name="m4567", tag="mt")
            nc.vector.tensor_max(m4567, m45, m67)
            m = mpool.tile([P, W], fp16, name="m", tag="m")
            nc.vector.tensor_max(m, m0123, m4567)

            # ---- per-class pred masks + PE tp/count matmuls ----
            for c in range(1, C):
                pred = maskp.tile([P, W], fp16, name=f"pred{c}", tag="pred")
                nc.vector.tensor_tensor(pred, chf[c], m, A.is_equal)
                gt = gts[c]
                for s in range(nsub):
                    first = pi == 0 and s == 0
                    last = pi == npieces - 1 and s == nsub - 1
                    nc.tensor.matmul(
                        psums[c - 1][:, :],
                        lhsT=pred[:, s * 128 : (s + 1) * 128],
                        rhs=gt[:, s * 128 : (s + 1) * 128],
                        start=first,
                        stop=last,
                    )
                    # cnts share one PSUM bank and start=True resets the
                    # WHOLE bank (measured): only the very first count
                    # matmul may use it; all later chains accumulate.
                    nc.tensor.matmul(
                        cnts[c - 1],
                        lhsT=pred[:, s * 128 : (s + 1) * 128],
                        rhs=ones[:, :],
                        start=first and c == 1,
                        stop=last and c == C - 1,
                        skip_group_check=True,
                    )

        for c in range(7):
            tps = accp.tile([P, 128], f32, name=f"tps{c}", tag=f"tps{c}")
            nc.scalar.copy(out=tps, in_=psums[c])
            nc.sync.dma_start(out=tp_out[c], in_=tps)
        pcs = accp.tile([P, 7], f32, name="pcs")
        nc.scalar.copy(out=pcs, in_=cntbank[:, 0:7])
        nc.sync.dma_start(out=pc_out[:, :], in_=pcs)

    nc.finalize()
    return nc


def _get_bass():
    global _CACHED_NC
    if _CACHED_NC is None:
        _CACHED_NC = build_bass()
    return _CACHED_NC


def make_in_maps(y_true, y_pred):
    yp = np.ascontiguousarray(np.asarray(y_pred, dtype=np.float32))
    yt = np.ascontiguousarray(np.asarray(y_true, dtype=np.int32))
    in_maps = []
    for i in range(N_CORES):
        yps = np.ascontiguousarray(yp[NB * i : NB * (i + 1)]).reshape(NB, C, NCHUNK, P, F)
        yts = np.ascontiguousarray(yt[NB * i : NB * (i + 1)]).reshape(NB, NCHUNK, P, F)
        in_maps.append({"yp": yps, "yt": yts})
    return in_maps


def epilogue(results, y_true):
    """Combine the 8 cores' partial sums into the final dice mean (float32,
    mirroring the reference arithmetic). gt counts come from the labels
    directly (exact)."""
    tp = np.zeros(7, dtype=np.float64)
    pred_cnt = np.zeros(7, dtype=np.float64)
    for r in results:
        tp += np.trace(np.asarray(r["tp_out"], dtype=np.float64), axis1=1, axis2=2)
        pred_cnt += np.asarray(r["pc_out"], dtype=np.float64).sum(axis=0)
    gt_cnt = np.bincount(
        np.asarray(y_true, dtype=np.int64).ravel(), minlength=8
    )[1:].astype(np.float64)

    # dice = (2tp + eps) / (2tp + fp + fn + eps), and
    # 2tp + fp + fn = pred_cnt + gt_cnt
    tp32 = tp.astype(np.float32)
    denom = (pred_cnt + gt_cnt).astype(np.float32)
    eps = np.float32(EPS)
    two = np.float32(2.0)
    dice = (two * tp32 + eps) / (denom + eps)
    return np.asarray(np.mean(dice, dtype=np.float32), dtype=np.float32)


def kernel(**inputs):
    from concourse.bass_utils import run_bass_kernel_spmd

    nc = _get_bass()
    in_maps = make_in_maps(inputs["y_true"], inputs["y_pred"])
    res = run_bass_kernel_spmd(nc, in_maps, core_ids=list(range(N_CORES)))
    return epilogue(res.results, inputs["y_true"])


if __name__ == "__main__":
    # smoke test with random data
    rng = np.random.default_rng(0)
    y_true = rng.integers(0, C, size=(16, 512, 512)).astype(np.int32)
    y_pred = rng.standard_normal((16, C, 512, 512)).astype(np.float32)
    out = kernel(y_true=y_true, y_pred=y_pred)
    print("kernel output:", out)


# revision 38
# speedup vs baseline: 1.0791x; 1.0791x over previous
"""Trainium2 Bass kernel for DiceLoss (hard-argmax dice, ignore background, mean).

Problem (hardcoded shapes):
  y_true: [16, 512, 512] int32 in [0, 8)
  y_pred: [16, 8, 512, 512] float32
  out   : scalar float32 = mean over classes 1..7 of
          (2*tp + eps) / (2*tp + fp + fn + eps)
  with pred_cls = argmax_c y_pred, one-hot tp/fp/fn sums over all pixels.
  Note 2*tp + fp + fn == pred_cnt + gt_cnt, so per class we only need
  tp, pred_cnt (both from the device) and gt_cnt (host bincount of y_true).

Strategy (8 NeuronCores, data-parallel over batch; measured-on-HW numbers in
brackets):
  - Each core processes 2 of the 16 batch images (SPMD, same NEFF), streamed
    in [128, 1024] chunks; the final chunk is split in two [128, 512] halves
    to shorten the post-DMA tail.
  - ScalarE converts the 8 channel planes fp32->fp16 and the label plane
    int32->fp16 [1.15us per [128,1024] op, no 16-bit speedup]. GpSimd is NOT
    used: it shares the DVE datapath and stalls DVE 2x/4x ops (measured).
  - VectorE (DVE): everything fp16 to hit the hardware perf modes
    [tensor_tensor 16-bit in+out runs 2x (~600ns); tensor_scalar 16-bit
    without accum_out runs 4x (~335ns); accum_out forces 1x so is avoided;
    scalar_tensor_tensor is always 1x so is avoided]:
      * 7-op pairwise-max tree over the fp16 channels -> m
      * pred_c = tensor_tensor is_equal(ch_fp16[c], m)   (2x)
      * gt_c   = tensor_scalar is_equal(lab_fp16, c)     (4x)
    fp16 keeps argmax-tie inflation ~0.05% of pixels (rel err ~3e-4,
    tolerance is 2e-2).
  - TensorE (PE): per class, tp via diag(pred_c^T @ gt_c) accumulated in a
    [128,128] PSUM bank over subtiles+chunks, plus pred_cnt via an extra
    rhs=ones[128,1] matmul on the already-loaded pred_c weights [LDWEIGHTS
    and MATMUL pipeline on separate units]. The 7 pred-count accumulators
    share the 8th PSUM bank; start=True resets the WHOLE bank (measured),
    so only the very first count matmul sets it.
  - Host: gt counts from np.bincount(y_true) (exact, input-only), then the
    dice mean in float32 mirroring the reference arithmetic.
"""

import numpy as np

EPS = 1e-05

# Problem geometry (hardcoded per the harness contract).
N_CORES = 8
NB = 2          # batch images per core
C = 8           # classes
P = 128         # SBUF partitions
F = 1024        # free-dim elements per chunk
NCHUNK = 2      # chunks per image plane (512*512 = 2*128*1024)

_CACHED_NC = None


def build_bass():
    """Build the Bass kernel (same NEFF for all 8 cores)."""
    from contextlib import ExitStack

    import concourse.bacc as bacc
    import concourse.tile as tile
    from concourse import mybir

    nc = bacc.Bacc(None, target_bir_lowering=False)
    f32 = mybir.dt.float32
    fp16 = mybir.dt.float16
    i32 = mybir.dt.int32
    A = mybir.AluOpType

    yp = nc.dram_tensor(
        "yp", [NB, C, NCHUNK, P, F], f32, kind="ExternalInput"
    )
    yt = nc.dram_tensor("yt", [NB, NCHUNK, P, F], i32, kind="ExternalInput")
    # identity used to extract PSUM diagonals on-device (np.eye from host)
    ident = nc.dram_tensor("ident", [P, 128], f32, kind="ExternalInput")
    # per-partition tp diagonal partials per class; host sums partitions.
    tp_out = nc.dram_tensor("tp_out", [P, 7], f32, kind="ExternalOutput")
    # pred counts: per class a [128, 1] PSUM accumulator; host sums partitions.
    pc_out = nc.dram_tensor("pc_out", [P, 7], f32, kind="ExternalOutput")

    # pieces: (n, j, lo, hi); final chunk split in half for a shorter tail
    pieces = []
    for n in range(NB):
        for j in range(NCHUNK):
            if n == NB - 1 and j == NCHUNK - 1:
                pieces.append((n, j, 0, F // 2))
                pieces.append((n, j, F // 2, F))
            else:
                pieces.append((n, j, 0, F))

    with tile.TileContext(nc) as tc, ExitStack() as ctx:
        chpool = ctx.enter_context(tc.tile_pool(name="ch", bufs=2))
        hpool = ctx.enter_context(tc.tile_pool(name="h", bufs=2))
        tpool = ctx.enter_context(tc.tile_pool(name="tt", bufs=2))
        mpool = ctx.enter_context(tc.tile_pool(name="mx", bufs=2))
        mtmp = ctx.enter_context(tc.tile_pool(name="mtmp", bufs=6))
        maskp = ctx.enter_context(tc.tile_pool(name="mask", bufs=3))
        gtpool = ctx.enter_context(tc.tile_pool(name="gt", bufs=9))
        constp = ctx.enter_context(tc.tile_pool(name="const", bufs=1))
        accp = ctx.enter_context(tc.tile_pool(name="acc", bufs=1))
        psump = ctx.enter_context(tc.tile_pool(name="psum", bufs=1, space="PSUM"))

        ones = constp.tile([P, 1], fp16, name="ones")
        nc.vector.memset(ones, 1.0)
        identt = constp.tile([P, 128], f32, name="identt")
        nc.sync.dma_start(out=identt, in_=ident[:, :])

        psums = [
            psump.tile([P, 128], f32, name=f"ps{c}", tag=f"ps{c}")
            for c in range(1, C)
        ]
        # all 7 pred-count accumulators share one PSUM bank (disjoint columns)
        cntbank = psump.tile([P, 8], f32, name="cntbank", tag="cntbank")
        cnts = [cntbank[:, c - 1 : c] for c in range(1, C)]

        npieces = len(pieces)
        for pi, (n, j, lo, hi) in enumerate(pieces):
            W = hi - lo
            nsub = W // 128
            # label DMA first so its convert (also first on ScalarE) never
            # stalls; the gt masks then run on DVE while channels stream
            tt = tpool.tile([P, W], i32, name="t", tag="t")
            nc.sync.dma_start(out=tt, in_=yt[n, j][:, lo:hi])
            ch = []
            for c in range(C):
                tl = chpool.tile([P, W], f32, name=f"ch{c}", tag=f"ch{c}")
                nc.sync.dma_start(out=tl, in_=yp[n, c, j][:, lo:hi])
                ch.append(tl)

            tf = tpool.tile([P, W], fp16, name="tf", tag="tf")
            nc.scalar.copy(out=tf, in_=tt)
            chf = []
            for c in range(C):
                tl = hpool.tile([P, W], fp16, name=f"hf{c}", tag=f"hf{c}")
                nc.scalar.copy(out=tl, in_=ch[c])
                chf.append(tl)

            # gt masks early: only need the labels (tensor_scalar 4x mode)
            gts = {}
            for c in range(1, C):
                gt = gtpool.tile([P, W], fp16, name=f"gt{c}", tag="gt")
                nc.vector.tensor_scalar(gt, tf, float(c), None, A.is_equal)
                gts[c] = gt

            # ---- max tree (DVE tensor_tensor fp16: 2x perf mode); ordered
            # so only m67 -> m4567 -> m depend on the last channel ----
            m01 = mtmp.tile([P, W], fp16, name="m01", tag="mt")
            nc.vector.tensor_max(m01, chf[0], chf[1])
            m23 = mtmp.tile([P, W], fp16, name="m23", tag="mt")
            nc.vector.tensor_max(m23, chf[2], chf[3])
            m0123 = mtmp.tile([P, W], fp16, name="m0123", tag="mt")
            nc.vector.tensor_max(m0123, m01, m23)
            m45 = mtmp.tile([P, W], fp16, name="m45", tag="mt")
            nc.vector.tensor_max(m45, chf[4], chf[5])
            m67 = mtmp.tile([P, W], fp16, name="m67", tag="mt")
            nc.vector.tensor_max(m67, chf[6], chf[7])
            m4567 = mtmp.tile([P, W], fp16, name="m4567", tag="mt")
            nc.vector.tensor_max(m4567, m45, m67)
            m = mpool.tile([P, W], fp16, name="m", tag="m")
            nc.vector.tensor_max(m, m0123, m4567)

            # ---- per-class pred masks + PE tp/count matmuls ----
            for c in range(1, C):
                pred = maskp.tile([P, W], fp16, name=f"pred{c}", tag="pred")
                nc.vector.tensor_tensor(pred, chf[c], m, A.is_equal)
                gt = gts[c]
                for s in range(nsub):
                    first = pi == 0 and s == 0
                    last = pi == npieces - 1 and s == nsub - 1
                    nc.tensor.matmul(
                        psums[c - 1][:, :],
                        lhsT=pred[:, s * 128 : (s + 1) * 128],
                        rhs=gt[:, s * 128 : (s + 1) * 128],
                        start=first,
                        stop=last,
                    )
                    # cnts share one PSUM bank and start=True resets the
                    # WHOLE bank (measured): only the very first count
                    # matmul may use it; all later chains accumulate.
                    nc.tensor.matmul(
                        cnts[c - 1],
                        lhsT=pred[:, s * 128 : (s + 1) * 128],
                        rhs=ones[:, :],
                        start=first and c == 1,
                        stop=last and c == C - 1,
                        skip_group_check=True,
                    )

        # drain: extract each tp PSUM diagonal via (psum * I) with accum_out
        # (per-partition sum) — output shrinks from 448KB to 3.5KB
        tpd = accp.tile([P, 7], f32, name="tpd")
        scratch = accp.tile([P, 128], f32, name="scratch")
        for c in range(7):
            nc.vector.scalar_tensor_tensor(
                scratch, psums[c], 1.0, identt, A.mult, A.mult,
                accum_out=tpd[:, c : c + 1],
            )
        nc.sync.dma_start(out=tp_out[:, :], in_=tpd)
        pcs = accp.tile([P, 7], f32, name="pcs")
        nc.scalar.copy(out=pcs, in_=cntbank[:, 0:7])
        nc.sync.dma_start(out=pc_out[:, :], in_=pcs)

    nc.finalize()
    return nc


def _get_bass():
    global _CACHED_NC
    if _CACHED_NC is None:
        _CACHED_NC = build_bass()
    return _CACHED_NC


def make_in_maps(y_true, y_pred):
    yp = np.ascontiguousarray(np.asarray(y_pred, dtype=np.float32))
    yt = np.ascontiguousarray(np.asarray(y_true, dtype=np.int32))
    eye = np.eye(P, dtype=np.float32)
    in_maps = []
    for i in range(N_CORES):
        yps = np.ascontiguousarray(yp[NB * i : NB * (i + 1)]).reshape(NB, C, NCHUNK, P, F)
        yts = np.ascontiguousarray(yt[NB * i : NB * (i + 1)]).reshape(NB, NCHUNK, P, F)
        in_maps.append({"yp": yps, "yt": yts, "ident": eye})
    return in_maps


def epilogue(results, y_true):
    """Combine the 8 cores' partial sums into the final dice mean (float32,
    mirroring the reference arithmetic). gt counts come from the labels
    directly (exact)."""
    tp = np.zeros(7, dtype=np.float64)
    pred_cnt = np.zeros(7, dtype=np.float64)
    for r in results:
        tp += np.asarray(r["tp_out"], dtype=np.float64).sum(axis=0)
        pred_cnt += np.asarray(r["pc_out"], dtype=np.float64).sum(axis=0)
    gt_cnt = np.bincount(
        np.asarray(y_true, dtype=np.int64).ravel(), minlength=8
    )[1:].astype(np.float64)

    # dice = (2tp + eps) / (2tp + fp + fn + eps), and
    # 2tp + fp + fn = pred_cnt + gt_cnt
    tp32 = tp.astype(np.float32)
    denom = (pred_cnt + gt_cnt).astype(np.float32)
    eps = np.float32(EPS)
    two = np.float32(2.0)
    dice = (two * tp32 + eps) / (denom + eps)
    return np.asarray(np.mean(dice, dtype=np.float32), dtype=np.float32)


def kernel(**inputs):
    from concourse.bass_utils import run_bass_kernel_spmd

    nc = _get_bass()
    in_maps = make_in_maps(inputs["y_true"], inputs["y_pred"])
    res = run_bass_kernel_spmd(nc, in_maps, core_ids=list(range(N_CORES)))
    return epilogue(res.results, inputs["y_true"])


if __name__ == "__main__":
    # smoke test with random data
    rng = np.random.default_rng(0)
    y_true = rng.integers(0, C, size=(16, 512, 512)).astype(np.int32)
    y_pred = rng.standard_normal((16, C, 512, 512)).astype(np.float32)
    out = kernel(y_true=y_true, y_pred=y_pred)
    print("kernel output:", out)


# revision 39
# speedup vs baseline: 1.0839x; 1.0044x over previous
"""Trainium2 Bass kernel for DiceLoss (hard-argmax dice, ignore background, mean).

Problem (hardcoded shapes):
  y_true: [16, 512, 512] int32 in [0, 8)
  y_pred: [16, 8, 512, 512] float32
  out   : scalar float32 = mean over classes 1..7 of
          (2*tp + eps) / (2*tp + fp + fn + eps)
  with pred_cls = argmax_c y_pred, one-hot tp/fp/fn sums over all pixels.
  Note 2*tp + fp + fn == pred_cnt + gt_cnt, so per class we only need
  tp, pred_cnt (both from the device) and gt_cnt (host bincount of y_true).

Strategy (8 NeuronCores, data-parallel over batch; measured-on-HW numbers in
brackets):
  - Each core processes 2 of the 16 batch images (SPMD, same NEFF), streamed
    in [128, 1024] chunks; the final chunk is split in two [128, 512] halves
    to shorten the post-DMA tail.
  - ScalarE converts the 8 channel planes fp32->fp16 and the label plane
    int32->fp16 [1.15us per [128,1024] op, no 16-bit speedup]. GpSimd is NOT
    used: it shares the DVE datapath and stalls DVE 2x/4x ops (measured).
  - VectorE (DVE): everything fp16 to hit the hardware perf modes
    [tensor_tensor 16-bit in+out runs 2x (~600ns); tensor_scalar 16-bit
    without accum_out runs 4x (~335ns); accum_out forces 1x so is avoided;
    scalar_tensor_tensor is always 1x so is avoided]:
      * 7-op pairwise-max tree over the fp16 channels -> m
      * pred_c = tensor_tensor is_equal(ch_fp16[c], m)   (2x)
      * gt_c   = tensor_scalar is_equal(lab_fp16, c)     (4x)
    fp16 keeps argmax-tie inflation ~0.05% of pixels (rel err ~3e-4,
    tolerance is 2e-2).
  - TensorE (PE): per class, tp via diag(pred_c^T @ gt_c) accumulated in a
    [128,128] PSUM bank over subtiles+chunks, plus pred_cnt via an extra
    rhs=ones[128,1] matmul on the already-loaded pred_c weights [LDWEIGHTS
    and MATMUL pipeline on separate units]. The 7 pred-count accumulators
    share the 8th PSUM bank; start=True resets the WHOLE bank (measured),
    so only the very first count matmul sets it.
  - Host: gt counts from np.bincount(y_true) (exact, input-only), then the
    dice mean in float32 mirroring the reference arithmetic.
"""

import numpy as np

EPS = 1e-05

# Problem geometry (hardcoded per the harness contract).
N_CORES = 8
NB = 2          # batch images per core
C = 8           # classes
P = 128         # SBUF partitions
F = 1024        # free-dim elements per chunk
NCHUNK = 2      # chunks per image plane (512*512 = 2*128*1024)

_CACHED_NC = None


def build_bass():
    """Build the Bass kernel (same NEFF for all 8 cores)."""
    from contextlib import ExitStack

    import concourse.bacc as bacc
    import concourse.tile as tile
    from concourse import mybir

    nc = bacc.Bacc(None, target_bir_lowering=False)
    f32 = mybir.dt.float32
    fp16 = mybir.dt.float16
    i32 = mybir.dt.int32
    A = mybir.AluOpType

    yp = nc.dram_tensor(
        "yp", [NB, C, NCHUNK, P, F], f32, kind="ExternalInput"
    )
    yt = nc.dram_tensor("yt", [NB, NCHUNK, P, F], i32, kind="ExternalInput")
    # tp partials: per class a [128, 128] PSUM accumulator; host takes trace().
    tp_out = nc.dram_tensor("tp_out", [7, P, 128], f32, kind="ExternalOutput")
    # pred counts: per class a [128, 1] PSUM accumulator; host sums partitions.
    pc_out = nc.dram_tensor("pc_out", [P, 7], f32, kind="ExternalOutput")

    # pieces: (n, j, lo, hi); final chunk split in half for a shorter tail
    pieces = []
    for n in range(NB):
        for j in range(NCHUNK):
            if n == NB - 1 and j == NCHUNK - 1:
                pieces.append((n, j, 0, F // 2))
                pieces.append((n, j, F // 2, F))
            else:
                pieces.append((n, j, 0, F))

    with tile.TileContext(nc) as tc, ExitStack() as ctx:
        chpool = ctx.enter_context(tc.tile_pool(name="ch", bufs=2))
        hpool = ctx.enter_context(tc.tile_pool(name="h", bufs=2))
        tpool = ctx.enter_context(tc.tile_pool(name="tt", bufs=2))
        mpool = ctx.enter_context(tc.tile_pool(name="mx", bufs=2))
        mtmp = ctx.enter_context(tc.tile_pool(name="mtmp", bufs=6))
        maskp = ctx.enter_context(tc.tile_pool(name="mask", bufs=3))
        gtpool = ctx.enter_context(tc.tile_pool(name="gt", bufs=9))
        constp = ctx.enter_context(tc.tile_pool(name="const", bufs=1))
        accp = ctx.enter_context(tc.tile_pool(name="acc", bufs=1))
        psump = ctx.enter_context(tc.tile_pool(name="psum", bufs=1, space="PSUM"))

        ones = constp.tile([P, 1], fp16, name="ones")
        nc.vector.memset(ones, 1.0)

        psums = [
            psump.tile([P, 128], f32, name=f"ps{c}", tag=f"ps{c}")
            for c in range(1, C)
        ]
        # all 7 pred-count accumulators share one PSUM bank (disjoint columns)
        cntbank = psump.tile([P, 8], f32, name="cntbank", tag="cntbank")
        cnts = [cntbank[:, c - 1 : c] for c in range(1, C)]

        npieces = len(pieces)
        for pi, (n, j, lo, hi) in enumerate(pieces):
            W = hi - lo
            nsub = W // 128
            # label DMA first so its convert (also first on ScalarE) never
            # stalls; the gt masks then run on DVE while channels stream
            tt = tpool.tile([P, W], i32, name="t", tag="t")
            nc.sync.dma_start(out=tt, in_=yt[n, j][:, lo:hi])
            ch = []
            for c in range(C):
                tl = chpool.tile([P, W], f32, name=f"ch{c}", tag=f"ch{c}")
                nc.sync.dma_start(out=tl, in_=yp[n, c, j][:, lo:hi])
                ch.append(tl)

            tf = tpool.tile([P, W], fp16, name="tf", tag="tf")
            nc.scalar.copy(out=tf, in_=tt)
            chf = []
            for c in range(C):
                tl = hpool.tile([P, W], fp16, name=f"hf{c}", tag=f"hf{c}")
                nc.scalar.copy(out=tl, in_=ch[c])
                chf.append(tl)

            # gt masks early: only need the labels (tensor_scalar 4x mode)
            gts = {}
            for c in range(1, C):
                gt = gtpool.tile([P, W], fp16, name=f"gt{c}", tag="gt")
                nc.vector.tensor_scalar(gt, tf, float(c), None, A.is_equal)
                gts[c] = gt

            # ---- max tree (DVE tensor_tensor fp16: 2x perf mode); ordered
            # so only m67 -> m4567 -> m depend on the last channel ----
            m01 = mtmp.tile([P, W], fp16, name="m01", tag="mt")
            nc.vector.tensor_max(m01, chf[0], chf[1])
            m23 = mtmp.tile([P, W], fp16, name="m23", tag="mt")
            nc.vector.tensor_max(m23, chf[2], chf[3])
            m0123 = mtmp.tile([P, W], fp16, name="m0123", tag="mt")
            nc.vector.tensor_max(m0123, m01, m23)
            m45 = mtmp.tile([P, W], fp16, name="m45", tag="mt")
            nc.vector.tensor_max(m45, chf[4], chf[5])
            m67 = mtmp.tile([P, W], fp16, name="m67", tag="mt")
            nc.vector.tensor_max(m67, chf[6], chf[7])
            m4567 = mtmp.tile([P, W], fp16, name="m4567", tag="mt")
            nc.vector.tensor_max(m4567, m45, m67)
            m = mpool.tile([P, W], fp16, name="m", tag="m")
            nc.vector.tensor_max(m, m0123, m4567)

            # ---- per-class pred masks + PE tp/count matmuls ----
            for c in range(1, C):
                pred = maskp.tile([P, W], fp16, name=f"pred{c}", tag="pred")
                nc.vector.tensor_tensor(pred, chf[c], m, A.is_equal)
                gt = gts[c]
                for s in range(nsub):
                    first = pi == 0 and s == 0
                    last = pi == npieces - 1 and s == nsub - 1
                    nc.tensor.matmul(
                        psums[c - 1][:, :],
                        lhsT=pred[:, s * 128 : (s + 1) * 128],
                        rhs=gt[:, s * 128 : (s + 1) * 128],
                        start=first,
                        stop=last,
                    )
                    # cnts share one PSUM bank and start=True resets the
                    # WHOLE bank (measured): only the very first count
                    # matmul may use it; all later chains accumulate.
                    nc.tensor.matmul(
                        cnts[c - 1],
                        lhsT=pred[:, s * 128 : (s + 1) * 128],
                        rhs=ones[:, :],
                        start=first and c == 1,
                        stop=last and c == C - 1,
                        skip_group_check=True,
                    )

        for c in range(7):
            tps = accp.tile([P, 128], f32, name=f"tps{c}", tag=f"tps{c}")
            nc.scalar.copy(out=tps, in_=psums[c])
            nc.sync.dma_start(out=tp_out[c], in_=tps)
        pcs = accp.tile([P, 7], f32, name="pcs")
        nc.scalar.copy(out=pcs, in_=cntbank[:, 0:7])
        nc.sync.dma_start(out=pc_out[:, :], in_=pcs)

    nc.finalize()
    return nc


def _get_bass():
    global _CACHED_NC
    if _CACHED_NC is None:
        _CACHED_NC = build_bass()
    return _CACHED_NC


def make_in_maps(y_true, y_pred):
    yp = np.ascontiguousarray(np.asarray(y_pred, dtype=np.float32))
    yt = np.ascontiguousarray(np.asarray(y_true, dtype=np.int32))
    in_maps = []
    for i in range(N_CORES):
        yps = np.ascontiguousarray(yp[NB * i : NB * (i + 1)]).reshape(NB, C, NCHUNK, P, F)
        yts = np.ascontiguousarray(yt[NB * i : NB * (i + 1)]).reshape(NB, NCHUNK, P, F)
        in_maps.append({"yp": yps, "yt": yts})
    return in_maps


def epilogue(results, y_true):
    """Combine the 8 cores' partial sums into the final dice mean (float32,
    mirroring the reference arithmetic). gt counts come from the labels
    directly (exact)."""
    tp = np.zeros(7, dtype=np.float64)
    pred_cnt = np.zeros(7, dtype=np.float64)
    for r in results:
        tp += np.trace(np.asarray(r["tp_out"], dtype=np.float64), axis1=1, axis2=2)
        pred_cnt += np.asarray(r["pc_out"], dtype=np.float64).sum(axis=0)
    gt_cnt = np.bincount(
        np.asarray(y_true, dtype=np.int64).ravel(), minlength=8
    )[1:].astype(np.float64)

    # dice = (2tp + eps) / (2tp + fp + fn + eps), and
    # 2tp + fp + fn = pred_cnt + gt_cnt
    tp32 = tp.astype(np.float32)
    denom = (pred_cnt + gt_cnt).astype(np.float32)
    eps = np.float32(EPS)
    two = np.float32(2.0)
    dice = (two * tp32 + eps) / (denom + eps)
    return np.asarray(np.mean(dice, dtype=np.float32), dtype=np.float32)


def kernel(**inputs):
    from concourse.bass_utils import run_bass_kernel_spmd

    nc = _get_bass()
    in_maps = make_in_maps(inputs["y_true"], inputs["y_pred"])
    res = run_bass_kernel_spmd(nc, in_maps, core_ids=list(range(N_CORES)))
    return epilogue(res.results, inputs["y_true"])


if __name__ == "__main__":
    # smoke test with random data
    rng = np.random.default_rng(0)
    y_true = rng.integers(0, C, size=(16, 512, 512)).astype(np.int32)
    y_pred = rng.standard_normal((16, C, 512, 512)).astype(np.float32)
    out = kernel(y_true=y_true, y_pred=y_pred)
    print("kernel output:", out)


# revision 40
# speedup vs baseline: 1.1442x; 1.0557x over previous
"""Trainium2 Bass kernel for DiceLoss (hard-argmax dice, ignore background, mean).

Problem (hardcoded shapes):
  y_true: [16, 512, 512] int32 in [0, 8)
  y_pred: [16, 8, 512, 512] float32
  out   : scalar float32 = mean over classes 1..7 of
          (2*tp + eps) / (2*tp + fp + fn + eps)
  with pred_cls = argmax_c y_pred, one-hot tp/fp/fn sums over all pixels.
  Note 2*tp + fp + fn == pred_cnt + gt_cnt, so per class we only need
  tp, pred_cnt (both from the device) and gt_cnt (host bincount of y_true).

Strategy (8 NeuronCores, data-parallel over batch; measured-on-HW numbers in
brackets):
  - Each core processes 2 of the 16 batch images (SPMD, same NEFF), streamed
    in [128, 1024] chunks; the final chunk is split in two [128, 512] halves
    to shorten the post-DMA tail.
  - ScalarE converts the 8 channel planes fp32->fp16 and the label plane
    int32->fp16 [1.15us per [128,1024] op, no 16-bit speedup]. GpSimd is NOT
    used: it shares the DVE datapath and stalls DVE 2x/4x ops (measured).
  - VectorE (DVE): everything fp16 to hit the hardware perf modes
    [tensor_tensor 16-bit in+out runs 2x (~600ns); tensor_scalar 16-bit
    without accum_out runs 4x (~335ns); accum_out forces 1x so is avoided;
    scalar_tensor_tensor is always 1x so is avoided]:
      * 7-op pairwise-max tree over the fp16 channels -> m
      * pred_c = tensor_tensor is_equal(ch_fp16[c], m)   (2x)
      * gt_c   = tensor_scalar is_equal(lab_fp16, c)     (4x)
    fp16 keeps argmax-tie inflation ~0.05% of pixels (rel err ~3e-4,
    tolerance is 2e-2).
  - TensorE (PE): per class, tp via diag(pred_c^T @ gt_c) accumulated in a
    [128,128] PSUM bank over subtiles+chunks, plus pred_cnt via an extra
    rhs=ones[128,1] matmul on the already-loaded pred_c weights [LDWEIGHTS
    and MATMUL pipeline on separate units]. The 7 pred-count accumulators
    share the 8th PSUM bank; start=True resets the WHOLE bank (measured),
    so only the very first count matmul sets it.
  - Host: gt counts from np.bincount(y_true) (exact, input-only), then the
    dice mean in float32 mirroring the reference arithmetic.
"""

import numpy as np

EPS = 1e-05

# Problem geometry (hardcoded per the harness contract).
N_CORES = 8
NB = 2          # batch images per core
C = 8           # classes
P = 128         # SBUF partitions
F = 1024        # free-dim elements per chunk
NCHUNK = 2      # chunks per image plane (512*512 = 2*128*1024)

_CACHED_NC = None


def build_bass():
    """Build the Bass kernel (same NEFF for all 8 cores)."""
    from contextlib import ExitStack

    import concourse.bacc as bacc
    import concourse.tile as tile
    from concourse import mybir

    nc = bacc.Bacc(None, target_bir_lowering=False)
    f32 = mybir.dt.float32
    fp16 = mybir.dt.float16
    i32 = mybir.dt.int32
    A = mybir.AluOpType

    yp = nc.dram_tensor(
        "yp", [NB, C, NCHUNK, P, F], f32, kind="ExternalInput"
    )
    yt = nc.dram_tensor("yt", [NB, NCHUNK, P, F], i32, kind="ExternalInput")
    # tp partials: per class a [128, 128] PSUM accumulator; host takes trace().
    tp_out = nc.dram_tensor("tp_out", [7, P, 128], f32, kind="ExternalOutput")
    # pred counts: per class a [128, 1] PSUM accumulator; host sums partitions.
    pc_out = nc.dram_tensor("pc_out", [P, 7], f32, kind="ExternalOutput")

    # pieces: (n, j, lo, hi)
    pieces = [(n, j, 0, F) for n in range(NB) for j in range(NCHUNK)]

    with tile.TileContext(nc) as tc, ExitStack() as ctx:
        chpool = ctx.enter_context(tc.tile_pool(name="ch", bufs=2))
        hpool = ctx.enter_context(tc.tile_pool(name="h", bufs=2))
        tpool = ctx.enter_context(tc.tile_pool(name="tt", bufs=2))
        mpool = ctx.enter_context(tc.tile_pool(name="mx", bufs=2))
        mtmp = ctx.enter_context(tc.tile_pool(name="mtmp", bufs=6))
        maskp = ctx.enter_context(tc.tile_pool(name="mask", bufs=5))
        gtpool = ctx.enter_context(tc.tile_pool(name="gt", bufs=9))
        constp = ctx.enter_context(tc.tile_pool(name="const", bufs=1))
        accp = ctx.enter_context(tc.tile_pool(name="acc", bufs=1))
        psump = ctx.enter_context(tc.tile_pool(name="psum", bufs=1, space="PSUM"))

        ones = constp.tile([P, 1], fp16, name="ones")
        nc.vector.memset(ones, 1.0)

        psums = [
            psump.tile([P, 128], f32, name=f"ps{c}", tag=f"ps{c}")
            for c in range(1, C)
        ]
        # all 7 pred-count accumulators share one PSUM bank (disjoint columns)
        cntbank = psump.tile([P, 8], f32, name="cntbank", tag="cntbank")
        cnts = [cntbank[:, c - 1 : c] for c in range(1, C)]

        npieces = len(pieces)
        for pi, (n, j, lo, hi) in enumerate(pieces):
            W = hi - lo
            nsub = W // 128
            # label DMA first so its convert (also first on ScalarE) never
            # stalls; the gt masks then run on DVE while channels stream
            tt = tpool.tile([P, W], i32, name="t", tag="t")
            nc.sync.dma_start(out=tt, in_=yt[n, j][:, lo:hi])
            ch = []
            for c in range(C):
                tl = chpool.tile([P, W], f32, name=f"ch{c}", tag=f"ch{c}")
                nc.sync.dma_start(out=tl, in_=yp[n, c, j][:, lo:hi])
                ch.append(tl)

            tf = tpool.tile([P, W], fp16, name="tf", tag="tf")
            nc.scalar.copy(out=tf, in_=tt)
            chf = []
            for c in range(C):
                tl = hpool.tile([P, W], fp16, name=f"hf{c}", tag=f"hf{c}")
                nc.scalar.copy(out=tl, in_=ch[c])
                chf.append(tl)

            # gt masks early: only need the labels (tensor_scalar 4x mode)
            gts = {}
            for c in range(1, C):
                gt = gtpool.tile([P, W], fp16, name=f"gt{c}", tag="gt")
                nc.vector.tensor_scalar(gt, tf, float(c), None, A.is_equal)
                gts[c] = gt

            # ---- max tree (DVE tensor_tensor fp16: 2x perf mode); ordered
            # so only m67 -> m4567 -> m depend on the last channel ----
            m01 = mtmp.tile([P, W], fp16, name="m01", tag="mt")
            nc.vector.tensor_max(m01, chf[0], chf[1])
            m23 = mtmp.tile([P, W], fp16, name="m23", tag="mt")
            nc.vector.tensor_max(m23, chf[2], chf[3])
            m0123 = mtmp.tile([P, W], fp16, name="m0123", tag="mt")
            nc.vector.tensor_max(m0123, m01, m23)
            m45 = mtmp.tile([P, W], fp16, name="m45", tag="mt")
            nc.vector.tensor_max(m45, chf[4], chf[5])
            m67 = mtmp.tile([P, W], fp16, name="m67", tag="mt")
            nc.vector.tensor_max(m67, chf[6], chf[7])
            m4567 = mtmp.tile([P, W], fp16, name="m4567", tag="mt")
            nc.vector.tensor_max(m4567, m45, m67)
            m = mpool.tile([P, W], fp16, name="m", tag="m")
            nc.vector.tensor_max(m, m0123, m4567)

            # ---- per-class pred masks + PE tp/count matmuls ----
            for c in range(1, C):
                pred = maskp.tile([P, W], fp16, name=f"pred{c}", tag="pred")
                nc.vector.tensor_tensor(pred, chf[c], m, A.is_equal)
                gt = gts[c]
                for s in range(nsub):
                    first = pi == 0 and s == 0
                    last = pi == npieces - 1 and s == nsub - 1
                    nc.tensor.matmul(
                        psums[c - 1][:, :],
                        lhsT=pred[:, s * 128 : (s + 1) * 128],
                        rhs=gt[:, s * 128 : (s + 1) * 128],
                        start=first,
                        stop=last,
                    )
                    # cnts share one PSUM bank and start=True resets the
                    # WHOLE bank (measured): only the very first count
                    # matmul may use it; all later chains accumulate.
                    nc.tensor.matmul(
                        cnts[c - 1],
                        lhsT=pred[:, s * 128 : (s + 1) * 128],
                        rhs=ones[:, :],
                        start=first and c == 1,
                        stop=last and c == C - 1,
                        skip_group_check=True,
                    )

        for c in range(7):
            tps = accp.tile([P, 128], f32, name=f"tps{c}", tag=f"tps{c}")
            nc.scalar.copy(out=tps, in_=psums[c])
            nc.sync.dma_start(out=tp_out[c], in_=tps)
        pcs = accp.tile([P, 7], f32, name="pcs")
        nc.scalar.copy(out=pcs, in_=cntbank[:, 0:7])
        nc.sync.dma_start(out=pc_out[:, :], in_=pcs)

    nc.finalize()
    return nc


def _get_bass():
    global _CACHED_NC
    if _CACHED_NC is None:
        _CACHED_NC = build_bass()
    return _CACHED_NC


def make_in_maps(y_true, y_pred):
    yp = np.ascontiguousarray(np.asarray(y_pred, dtype=np.float32))
    yt = np.ascontiguousarray(np.asarray(y_true, dtype=np.int32))
    in_maps = []
    for i in range(N_CORES):
        yps = np.ascontiguousarray(yp[NB * i : NB * (i + 1)]).reshape(NB, C, NCHUNK, P, F)
        yts = np.ascontiguousarray(yt[NB * i : NB * (i + 1)]).reshape(NB, NCHUNK, P, F)
        in_maps.append({"yp": yps, "yt": yts})
    return in_maps


def epilogue(results, y_true):
    """Combine the 8 cores' partial sums into the final dice mean (float32,
    mirroring the reference arithmetic). gt counts come from the labels
    directly (exact)."""
    tp = np.zeros(7, dtype=np.float64)
    pred_cnt = np.zeros(7, dtype=np.float64)
    for r in results:
        tp += np.trace(np.asarray(r["tp_out"], dtype=np.float64), axis1=1, axis2=2)
        pred_cnt += np.asarray(r["pc_out"], dtype=np.float64).sum(axis=0)
    gt_cnt = np.bincount(
        np.asarray(y_true, dtype=np.int64).ravel(), minlength=8
    )[1:].astype(np.float64)

    # dice = (2tp + eps) / (2tp + fp + fn + eps), and
    # 2tp + fp + fn = pred_cnt + gt_cnt
    tp32 = tp.astype(np.float32)
    denom = (pred_cnt + gt_cnt).astype(np.float32)
    eps = np.float32(EPS)
    two = np.float32(2.0)
    dice = (two * tp32 + eps) / (denom + eps)
    return np.asarray(np.mean(dice, dtype=np.float32), dtype=np.float32)


def kernel(**inputs):
    from concourse.bass_utils import run_bass_kernel_spmd

    nc = _get_bass()
    in_maps = make_in_maps(inputs["y_true"], inputs["y_pred"])
    res = run_bass_kernel_spmd(nc, in_maps, core_ids=list(range(N_CORES)))
    return epilogue(res.results, inputs["y_true"])


if __name__ == "__main__":
    # smoke test with random data
    rng = np.random.default_rng(0)
    y_true = rng.integers(0, C, size=(16, 512, 512)).astype(np.int32)
    y_pred = rng.standard_normal((16, C, 512, 512)).astype(np.float32)
    out = kernel(y_true=y_true, y_pred=y_pred)
    print("kernel output:", out)
